# revision 2
# baseline (speedup 1.0000x reference)
"""FourierBlock Trainium2 kernel, v3.

Per core (= head h), three dense matmul stages with batch-half (16-batch)
pipelining so the mid-phase overlaps the in/out HBM streams:
  fwd:  psX[(ri,m), (b16,i)] = sum_l Fwd[l,(ri,m)] q[l,(b,i)]   (K=l, 16 chunks)
  T1:   psX -> xs (cast) -> xt[(ri,i), (b16,m)]   (DVE ri=0 + PE-transpose ri=1)
  mix:  psM[(ro,o), (m,b16)] = Wbig[m]^T xt[:, (.,m)]           per mode m
  T2:   psM -> os (cast) -> o2[(ri,m), (b16,o)]   (PE transposes)
  inv:  psY[(b2,o), l] = o2-chunk^T @ G2                        per 128-col chunk

The PE clock is kept unthrottled (HAM) with prewarm matmuls and warm-keepers
gated on DMA/DVE progress. Output evacuation rotates Vector/Scalar engines.
"""

import numpy as np
import ml_dtypes

import concourse.bacc as bacc
import concourse.mybir as mybir
import concourse.tile as tile
from concourse.bass_utils import run_bass_kernel_spmd

B, L, H, E, M = 32, 2048, 8, 64, 64
NCHUNK = L // 128           # 16 l-chunks of 128
BH = B // 2                 # 16 batches per half
W = BH * E                  # 1024 columns per half
BF16 = mybir.dt.bfloat16
F32 = mybir.dt.float32
NPBF16 = ml_dtypes.bfloat16

_PROGRAM = None


def _build_program():
    nc = bacc.Bacc(target_bir_lowering=False)

    qt = nc.dram_tensor("qt", [2, NCHUNK, 128, W], BF16, kind="ExternalInput")
    wb = nc.dram_tensor("wb", [128, M * 128], BF16, kind="ExternalInput")
    fwd = nc.dram_tensor("fwd", [128, NCHUNK * 128], BF16, kind="ExternalInput")
    g2 = nc.dram_tensor("g2", [128, L], BF16, kind="ExternalInput")
    ident = nc.dram_tensor("ident", [128, 64], BF16, kind="ExternalInput")
    yt = nc.dram_tensor("yt", [2, BH // 2, 128, L], BF16, kind="ExternalOutput")

    with tile.TileContext(nc) as tc:
        with (
            tc.tile_pool(name="const", bufs=1) as cpool,
            tc.tile_pool(name="qpool", bufs=1) as qpool,
            tc.tile_pool(name="work", bufs=1) as wpool,
            tc.tile_pool(name="yout", bufs=4) as ypool,
            tc.tile_pool(name="psq", bufs=1, space="PSUM") as psq,
            tc.tile_pool(name="psm", bufs=1, space="PSUM") as psm,
            tc.tile_pool(name="psy", bufs=2, space="PSUM") as psy,
        ):
            # ---- input DMAs ----
            # Phase 1 on the sync queue: fwd basis, q half-0 (per-chunk for
            # fine-grained matmul gating), then wb (4 slices; mix is gated
            # per-slice via subtile deps) and g2.  q half-1 is issued from
            # the SCALAR engine after the h0 cast so its transfers don't
            # steal HBM bandwidth from phase 1 (all active DMA queues share
            # the ~358 GB/s round-robin).
            fwd_sb = cpool.tile([128, NCHUNK * 128], BF16, tag="fwd")
            nc.sync.dma_start(out=fwd_sb[:], in_=fwd[:])
            ident_sb = cpool.tile([128, 64], BF16, tag="ident")
            nc.sync.dma_start(out=ident_sb[:], in_=ident[:])

            q_sb = [None, None]
            for h in range(2):
                q_sb[h] = qpool.tile(
                    [128, NCHUNK * W], BF16, tag=f"q{h}", name=f"q{h}"
                )

            def dma_q(h, eng):
                for c in range(NCHUNK):
                    eng.dma_start(
                        out=q_sb[h][:, c * W:(c + 1) * W],
                        in_=qt[h, c],
                    )

            dma_q(0, nc.sync)
            wb_sb = cpool.tile([128, M * 128], BF16, tag="wb")
            for s in range(4):
                nc.sync.dma_start(out=wb_sb[:, s * 2048:(s + 1) * 2048],
                                  in_=wb[:, s * 2048:(s + 1) * 2048])
            g2_sb = cpool.tile([128, L], BF16, tag="g2")
            nc.sync.dma_start(out=g2_sb[:], in_=g2[:])

            xt_sb = [None, None]
            o2_sb = [None, None]
            evac_n = [0]

            def warmmm(rhs_ap, n=1):
                # tiny matmuls that keep the PE HAM unthrottled; gated on
                # rhs_ap's producer so they spread out in time.
                k = rhs_ap.partition_size()
                b0 = rhs_ap.base_partition()
                nf = rhs_ap.free_size()
                for _ in range(n):
                    wps = psy.tile([128, 512], F32, tag="y", name="warm")
                    nc.tensor.matmul(wps[:, 0:nf], fwd_sb[b0:b0 + k, 0:128],
                                     rhs_ap, start=True, stop=True)

            def fwd_stage(h):
                psX = psq.tile([128, W], F32, tag="x", name=f"psX{h}")
                for c in range(NCHUNK):
                    if h == 0:
                        # warm chain: ungated junk BEFORE each gated chunk so
                        # the PE never idles >1us while q streams in
                        warmmm(fwd_sb[:, 0:512], n=2)
                    for j in range(2):
                        nc.tensor.matmul(
                            psX[:, j * 512:(j + 1) * 512],
                            fwd_sb[:, c * 128:(c + 1) * 128],
                            q_sb[h][:, c * W + j * 512: c * W + (j + 1) * 512],
                            start=(c == 0),
                            stop=(c == NCHUNK - 1),
                        )
                return psX

            def t1_stage(h, psX):
                # psX[(ri,m), (b,i)] --cast--> xs --> xt[(ri,i), (b,m)]
                # ri=0 rows on DVE (32x32 blocks), ri=1 rows on PE transpose
                xs = wpool.tile([128, W], BF16, tag=f"xs{h}", name=f"xs{h}")
                nc.scalar.copy(xs[:], psX[:])
                if h == 0:
                    # release the q half-1 stream only now: phase-1 DMAs
                    # (q h0 + wb + g2) get the full HBM bandwidth first
                    dma_q(1, nc.scalar)
                xt = wpool.tile([128, W], BF16, tag=f"xt{h}", name=f"xt{h}")
                xt_sb[h] = xt
                src = xs[:].rearrange("p (b i) -> p b i", i=E)
                dst = xt[:].rearrange("p (b m) -> p b m", m=E)
                # PE part: ri=1 (rows 64:128), 16 64x64 blocks
                psT1 = psm.tile([128, W], BF16, tag="m", name=f"psT1_{h}")
                for b in range(BH):
                    nc.tensor.transpose(
                        psT1[64:128, b * 64:(b + 1) * 64],
                        xs[64:128, b * 64:(b + 1) * 64],
                        ident_sb[64:128, :],
                    )
                # DVE part: ri=0 (rows 0:64), 4 strided 32x32 block calls
                for mh in range(2):
                    for ih in range(2):
                        nc.vector.transpose(
                            dst[ih * 32: ih * 32 + 32, :, mh * 32: mh * 32 + 32],
                            src[mh * 32: mh * 32 + 32, :, ih * 32: ih * 32 + 32],
                        )
                nc.vector.tensor_copy(xt[64:128, :], psT1[64:128, :])

            def mix_stage(h):
                # psM[(ro,o), (m,b)]: per-mode 16 contiguous cols (one bank)
                psM = psm.tile([128, W], F32, tag="m", name=f"psM{h}")
                xt_r = xt_sb[h][:].rearrange("p (b m) -> p m b", m=E)
                for m in range(M):
                    nc.tensor.matmul(
                        psM[:, m * BH:(m + 1) * BH],
                        wb_sb[:, m * 128:(m + 1) * 128],
                        xt_r[:, m, :],
                        start=True,
                        stop=True,
                    )
                return psM

            def t2_stage(h, psM):
                # psM[(ro,o), (m,b)] --cast--> os --PE transpose--> o2[(ri,m), (b,o)]
                os_ = wpool.tile([128, W], BF16, tag=f"os{h}", name=f"os{h}")
                nc.scalar.copy(os_[:], psM[:])
                o2 = wpool.tile([128, W], BF16, tag=f"o2{h}", name=f"o2{h}")
                o2_sb[h] = o2
                os_r = os_[:].rearrange("p (m b) -> p b m", b=BH)
                psT2 = psm.tile([128, W], BF16, tag="m", name=f"psT2_{h}")
                for ro in range(2):
                    for b in range(BH):
                        nc.tensor.transpose(
                            psT2[ro * 64:(ro + 1) * 64, b * 64:(b + 1) * 64],
                            os_r[ro * 64:(ro + 1) * 64, b, :],
                            ident_sb[ro * 64:(ro + 1) * 64, :],
                        )
                nc.vector.tensor_copy(o2[:], psT2[:])

            def inv_groups(h, ks):
                # psY[(b2,o), l] = o2[:, k-chunk]^T @ g2 ; evac + DMA out
                for k in ks:
                    lhsT = o2_sb[h][:, k * 128:(k + 1) * 128]
                    for lh in range(2):
                        psY = psy.tile([128, W], F32, tag="y",
                                       name=f"psY{h}_{k}_{lh}")
                        for j in range(2):
                            nc.tensor.matmul(
                                psY[:, j * 512:(j + 1) * 512],
                                lhsT,
                                g2_sb[:, lh * 1024 + j * 512:
                                      lh * 1024 + (j + 1) * 512],
                                start=True,
                                stop=True,
                            )
                        ysb = ypool.tile([128, W], BF16, tag="y",
                                         name=f"ysb{h}_{k}_{lh}")
                        t = evac_n[0]
                        evac_n[0] += 1
                        if t % 2 == 0:
                            nc.vector.tensor_copy(ysb[:], psY[:])
                        else:
                            nc.scalar.copy(ysb[:], psY[:])
                        nc.sync.dma_start(
                            out=yt[h, k, :, lh * 1024:(lh + 1) * 1024],
                            in_=ysb[:],
                        )

            # ---- PE program order (single in-order engine) ----
            for _ in range(16):  # prewarm: unthrottle HAM early
                warmmm(fwd_sb[:, 0:256], n=1)
            psX0 = fwd_stage(0)
            t1_stage(0, psX0)
            psM0 = mix_stage(0)
            t2_stage(0, psM0)
            psX1 = fwd_stage(1)
            t1_stage(1, psX1)
            inv_groups(0, range(0, 4))
            psM1 = mix_stage(1)
            inv_groups(0, range(4, 8))
            t2_stage(1, psM1)
            inv_groups(1, range(0, 8))

    nc.finalize()
    return nc


def _get_program():
    global _PROGRAM
    if _PROGRAM is None:
        _PROGRAM = _build_program()
    return _PROGRAM


def _host_prep(q, w_real, w_imag, index):
    q = np.asarray(q, dtype=np.float32)
    wr = np.asarray(w_real, dtype=np.float32)
    wi = np.asarray(w_imag, dtype=np.float32)
    index = np.asarray(index).astype(np.int64)

    qT = np.ascontiguousarray(q.transpose(2, 1, 0, 3))          # [H, L, B, E]
    qt4 = qT.reshape(H, NCHUNK, 128, 2, W).transpose(0, 3, 1, 2, 4)
    qt4 = np.ascontiguousarray(qt4).astype(NPBF16)              # [H, 2, 16, 128, W]

    wrT = wr.transpose(0, 1, 3, 2)                              # [h, i, m, o]
    wiT = wi.transpose(0, 1, 3, 2)
    A = np.empty((H, 128, M, 128), np.float32)
    A[:, :64, :, :64] = wrT
    A[:, :64, :, 64:] = wiT
    A[:, 64:, :, :64] = -wiT
    A[:, 64:, :, 64:] = wrT
    wb_np = A.reshape(H, 128, M * 128).astype(NPBF16)

    l = np.arange(L, dtype=np.float64)[:, None]
    ang = 2.0 * np.pi * index[None, :] * l / L                  # [L, M]
    F = np.concatenate([np.cos(ang), -np.sin(ang)], axis=1)     # [L, 2M]
    fwd_np = (
        F.reshape(NCHUNK, 128, 128).transpose(1, 0, 2).reshape(128, NCHUNK * 128)
    ).astype(NPBF16)

    mm = np.arange(M, dtype=np.float64)
    ang2 = 2.0 * np.pi * mm[:, None] * np.arange(L)[None, :] / L   # [M, L]
    c = np.where(mm == 0, 1.0, 2.0)[:, None] / L
    G2 = np.concatenate([c * np.cos(ang2), -c * np.sin(ang2)], axis=0)  # [2M, L]
    g2_np = G2.astype(NPBF16)

    ident_np = np.concatenate([np.eye(64), np.eye(64)], axis=0).astype(NPBF16)

    return qt4, wb_np, fwd_np, g2_np, ident_np


def run(inputs, trace=False):
    q = inputs["q"]
    qt4, wb_np, fwd_np, g2_np, ident_np = _host_prep(
        q, inputs["w_real"], inputs["w_imag"], inputs["index"]
    )
    nc = _get_program()
    in_maps = [
        {"qt": qt4[h], "wb": wb_np[h], "fwd": fwd_np, "g2": g2_np,
         "ident": ident_np}
        for h in range(H)
    ]
    res = run_bass_kernel_spmd(nc, in_maps, list(range(H)), trace=trace)
    arr = np.stack([res.results[h]["yt"] for h in range(H)])  # [H, 2, 8, 128, L]
    arr = arr.astype(np.float32).reshape(H, 2, BH // 2, 2, E, L)
    # p = (b2, o): b = half*16 + k*2 + b2 ; y[b, head, o, l]
    y = np.ascontiguousarray(
        arr.transpose(1, 2, 3, 0, 4, 5).reshape(B, H, E, L)
    ).astype(np.float32)
    return y, res


def kernel(**inputs) -> np.ndarray:
    y, _ = run(inputs, trace=False)
    return y


# revision 3
# speedup vs baseline: 1.1236x; 1.1236x over previous
"""FourierBlock Trainium2 kernel, v3.

Per core (= head h), three dense matmul stages with batch-half (16-batch)
pipelining so the mid-phase overlaps the in/out HBM streams:
  fwd:  psX[(ri,m), (b16,i)] = sum_l Fwd[l,(ri,m)] q[l,(b,i)]   (K=l, 16 chunks)
  T1:   psX -> xs (cast) -> xt[(ri,i), (b16,m)]   (DVE ri=0 + PE-transpose ri=1)
  mix:  psM[(ro,o), (m,b16)] = Wbig[m]^T xt[:, (.,m)]           per mode m
  T2:   psM -> os (cast) -> o2[(ri,m), (b16,o)]   (PE transposes)
  inv:  psY[(b2,o), l] = o2-chunk^T @ G2                        per 128-col chunk

The PE clock is kept unthrottled (HAM) with prewarm matmuls and warm-keepers
gated on DMA/DVE progress. Output evacuation rotates Vector/Scalar engines.
"""

import numpy as np
import ml_dtypes

import concourse.bacc as bacc
import concourse.mybir as mybir
import concourse.tile as tile
from concourse.bass_utils import run_bass_kernel_spmd

B, L, H, E, M = 32, 2048, 8, 64, 64
NCHUNK = L // 128           # 16 l-chunks of 128
BH = B // 2                 # 16 batches per half
W = BH * E                  # 1024 columns per half
BF16 = mybir.dt.bfloat16
F32 = mybir.dt.float32
NPBF16 = ml_dtypes.bfloat16

_PROGRAM = None


def _build_program():
    nc = bacc.Bacc(target_bir_lowering=False)

    qt = nc.dram_tensor("qt", [2, NCHUNK, 128, W], BF16, kind="ExternalInput")
    wb = nc.dram_tensor("wb", [128, M * 128], BF16, kind="ExternalInput")
    fwd = nc.dram_tensor("fwd", [128, NCHUNK * 128], BF16, kind="ExternalInput")
    g2 = nc.dram_tensor("g2", [128, L], BF16, kind="ExternalInput")
    ident = nc.dram_tensor("ident", [128, 64], BF16, kind="ExternalInput")
    yt = nc.dram_tensor("yt", [2, BH // 2, 128, L], BF16, kind="ExternalOutput")

    with tile.TileContext(nc) as tc:
        with (
            tc.tile_pool(name="const", bufs=1) as cpool,
            tc.tile_pool(name="qpool", bufs=1) as qpool,
            tc.tile_pool(name="work", bufs=1) as wpool,
            tc.tile_pool(name="yout", bufs=4) as ypool,
            tc.tile_pool(name="psq", bufs=1, space="PSUM") as psq,
            tc.tile_pool(name="psm", bufs=1, space="PSUM") as psm,
            tc.tile_pool(name="psy", bufs=2, space="PSUM") as psy,
        ):
            # ---- input DMAs (issue order ~ arrival order) ----
            fwd_sb = cpool.tile([128, NCHUNK * 128], BF16, tag="fwd")
            nc.sync.dma_start(out=fwd_sb[:], in_=fwd[:])
            ident_sb = cpool.tile([128, 64], BF16, tag="ident")
            nc.sync.dma_start(out=ident_sb[:], in_=ident[:])
            wb_sb = cpool.tile([128, M * 128], BF16, tag="wb")
            nc.sync.dma_start(out=wb_sb[:, 0:4096], in_=wb[:, 0:4096])
            nc.sync.dma_start(out=wb_sb[:, 4096:8192], in_=wb[:, 4096:8192])

            q_sb = [None, None]
            for h in range(2):
                q_sb[h] = qpool.tile(
                    [128, NCHUNK * W], BF16, tag=f"q{h}", name=f"q{h}"
                )

            def dma_q(h):
                for j in range(NCHUNK // 2):
                    nc.sync.dma_start(
                        out=q_sb[h][:, j * 2 * W:(j + 1) * 2 * W].rearrange(
                            "p (c f) -> p c f", c=2
                        ),
                        in_=qt[h, 2 * j:2 * j + 2].rearrange("c p f -> p c f"),
                    )

            dma_q(0)
            g2_sb = cpool.tile([128, L], BF16, tag="g2")
            nc.sync.dma_start(out=g2_sb[:], in_=g2[:])
            dma_q(1)

            xt_sb = [None, None]
            o2_sb = [None, None]
            evac_n = [0]

            def warmmm(rhs_ap, n=1):
                # tiny matmuls that keep the PE HAM unthrottled; gated on
                # rhs_ap's producer so they spread out in time.
                k = rhs_ap.partition_size()
                b0 = rhs_ap.base_partition()
                for _ in range(n):
                    wps = psy.tile([128, 512], F32, tag="y", name="warm")
                    nc.tensor.matmul(wps[:, 0:256], fwd_sb[b0:b0 + k, 0:128],
                                     rhs_ap, start=True, stop=True)

            def fwd_stage(h):
                psX = psq.tile([128, W], F32, tag="x", name=f"psX{h}")
                for c in range(NCHUNK):
                    for j in range(2):
                        nc.tensor.matmul(
                            psX[:, j * 512:(j + 1) * 512],
                            fwd_sb[:, c * 128:(c + 1) * 128],
                            q_sb[h][:, c * W + j * 512: c * W + (j + 1) * 512],
                            start=(c == 0),
                            stop=(c == NCHUNK - 1),
                        )
                    if h == 0 and c % 2 == 1:
                        # keep HAM busy during the DMA-paced in-stream
                        warmmm(q_sb[h][:, c * W: c * W + 256], n=2)
                return psX

            def t1_stage(h, psX):
                # psX[(ri,m), (b,i)] --cast--> xs --> xt[(ri,i), (b,m)]
                # ri=0 rows on DVE (32x32 blocks), ri=1 rows on PE transpose
                xs = wpool.tile([128, W], BF16, tag=f"xs{h}", name=f"xs{h}")
                nc.scalar.copy(xs[:], psX[:])
                xt = wpool.tile([128, W], BF16, tag=f"xt{h}", name=f"xt{h}")
                xt_sb[h] = xt
                src = xs[:].rearrange("p (b i) -> p b i", i=E)
                dst = xt[:].rearrange("p (b m) -> p b m", m=E)
                # PE part: ri=1 (rows 64:128), 16 64x64 blocks
                psT1 = psm.tile([128, W], BF16, tag="m", name=f"psT1_{h}")
                for b in range(BH):
                    nc.tensor.transpose(
                        psT1[64:128, b * 64:(b + 1) * 64],
                        xs[64:128, b * 64:(b + 1) * 64],
                        ident_sb[64:128, :],
                    )
                # DVE part: ri=0 (rows 0:64), 4 strided 32x32 block calls
                for mh in range(2):
                    for ih in range(2):
                        nc.vector.transpose(
                            dst[ih * 32: ih * 32 + 32, :, mh * 32: mh * 32 + 32],
                            src[mh * 32: mh * 32 + 32, :, ih * 32: ih * 32 + 32],
                        )
                nc.vector.tensor_copy(xt[64:128, :], psT1[64:128, :])

            def mix_stage(h):
                # psM[(ro,o), (m,b)]: per-mode 16 contiguous cols (one bank)
                psM = psm.tile([128, W], F32, tag="m", name=f"psM{h}")
                xt_r = xt_sb[h][:].rearrange("p (b m) -> p m b", m=E)
                for m in range(M):
                    nc.tensor.matmul(
                        psM[:, m * BH:(m + 1) * BH],
                        wb_sb[:, m * 128:(m + 1) * 128],
                        xt_r[:, m, :],
                        start=True,
                        stop=True,
                    )
                return psM

            def t2_stage(h, psM):
                # psM[(ro,o), (m,b)] --cast--> os --PE transpose--> o2[(ri,m), (b,o)]
                os_ = wpool.tile([128, W], BF16, tag=f"os{h}", name=f"os{h}")
                nc.scalar.copy(os_[:], psM[:])
                o2 = wpool.tile([128, W], BF16, tag=f"o2{h}", name=f"o2{h}")
                o2_sb[h] = o2
                os_r = os_[:].rearrange("p (m b) -> p b m", b=BH)
                psT2 = psm.tile([128, W], BF16, tag="m", name=f"psT2_{h}")
                for ro in range(2):
                    for b in range(BH):
                        nc.tensor.transpose(
                            psT2[ro * 64:(ro + 1) * 64, b * 64:(b + 1) * 64],
                            os_r[ro * 64:(ro + 1) * 64, b, :],
                            ident_sb[ro * 64:(ro + 1) * 64, :],
                        )
                nc.vector.tensor_copy(o2[:], psT2[:])

            def inv_groups(h, ks):
                # psY[(b2,o), l] = o2[:, k-chunk]^T @ g2 ; evac + DMA out
                for k in ks:
                    lhsT = o2_sb[h][:, k * 128:(k + 1) * 128]
                    for lh in range(2):
                        psY = psy.tile([128, W], F32, tag="y",
                                       name=f"psY{h}_{k}_{lh}")
                        for j in range(2):
                            nc.tensor.matmul(
                                psY[:, j * 512:(j + 1) * 512],
                                lhsT,
                                g2_sb[:, lh * 1024 + j * 512:
                                      lh * 1024 + (j + 1) * 512],
                                start=True,
                                stop=True,
                            )
                        ysb = ypool.tile([128, W], BF16, tag="y",
                                         name=f"ysb{h}_{k}_{lh}")
                        t = evac_n[0]
                        evac_n[0] += 1
                        if t % 2 == 0:
                            nc.vector.tensor_copy(ysb[:], psY[:])
                        else:
                            nc.scalar.copy(ysb[:], psY[:])
                        nc.sync.dma_start(
                            out=yt[h, k, :, lh * 1024:(lh + 1) * 1024],
                            in_=ysb[:],
                        )

            # ---- PE program order (single in-order engine) ----
            for _ in range(16):  # prewarm: unthrottle HAM early
                warmmm(fwd_sb[:, 0:256], n=1)
            psX0 = fwd_stage(0)
            t1_stage(0, psX0)
            psM0 = mix_stage(0)
            t2_stage(0, psM0)
            psX1 = fwd_stage(1)
            t1_stage(1, psX1)
            inv_groups(0, range(0, 4))
            psM1 = mix_stage(1)
            inv_groups(0, range(4, 8))
            t2_stage(1, psM1)
            inv_groups(1, range(0, 8))

    nc.finalize()
    return nc


def _get_program():
    global _PROGRAM
    if _PROGRAM is None:
        _PROGRAM = _build_program()
    return _PROGRAM


def _host_prep(q, w_real, w_imag, index):
    q = np.asarray(q, dtype=np.float32)
    wr = np.asarray(w_real, dtype=np.float32)
    wi = np.asarray(w_imag, dtype=np.float32)
    index = np.asarray(index).astype(np.int64)

    qT = np.ascontiguousarray(q.transpose(2, 1, 0, 3))          # [H, L, B, E]
    qt4 = qT.reshape(H, NCHUNK, 128, 2, W).transpose(0, 3, 1, 2, 4)
    qt4 = np.ascontiguousarray(qt4).astype(NPBF16)              # [H, 2, 16, 128, W]

    wrT = wr.transpose(0, 1, 3, 2)                              # [h, i, m, o]
    wiT = wi.transpose(0, 1, 3, 2)
    A = np.empty((H, 128, M, 128), np.float32)
    A[:, :64, :, :64] = wrT
    A[:, :64, :, 64:] = wiT
    A[:, 64:, :, :64] = -wiT
    A[:, 64:, :, 64:] = wrT
    wb_np = A.reshape(H, 128, M * 128).astype(NPBF16)

    l = np.arange(L, dtype=np.float64)[:, None]
    ang = 2.0 * np.pi * index[None, :] * l / L                  # [L, M]
    F = np.concatenate([np.cos(ang), -np.sin(ang)], axis=1)     # [L, 2M]
    fwd_np = (
        F.reshape(NCHUNK, 128, 128).transpose(1, 0, 2).reshape(128, NCHUNK * 128)
    ).astype(NPBF16)

    mm = np.arange(M, dtype=np.float64)
    ang2 = 2.0 * np.pi * mm[:, None] * np.arange(L)[None, :] / L   # [M, L]
    c = np.where(mm == 0, 1.0, 2.0)[:, None] / L
    G2 = np.concatenate([c * np.cos(ang2), -c * np.sin(ang2)], axis=0)  # [2M, L]
    g2_np = G2.astype(NPBF16)

    ident_np = np.concatenate([np.eye(64), np.eye(64)], axis=0).astype(NPBF16)

    return qt4, wb_np, fwd_np, g2_np, ident_np


def run(inputs, trace=False):
    q = inputs["q"]
    qt4, wb_np, fwd_np, g2_np, ident_np = _host_prep(
        q, inputs["w_real"], inputs["w_imag"], inputs["index"]
    )
    nc = _get_program()
    in_maps = [
        {"qt": qt4[h], "wb": wb_np[h], "fwd": fwd_np, "g2": g2_np,
         "ident": ident_np}
        for h in range(H)
    ]
    res = run_bass_kernel_spmd(nc, in_maps, list(range(H)), trace=trace)
    arr = np.stack([res.results[h]["yt"] for h in range(H)])  # [H, 2, 8, 128, L]
    arr = arr.astype(np.float32).reshape(H, 2, BH // 2, 2, E, L)
    # p = (b2, o): b = half*16 + k*2 + b2 ; y[b, head, o, l]
    y = np.ascontiguousarray(
        arr.transpose(1, 2, 3, 0, 4, 5).reshape(B, H, E, L)
    ).astype(np.float32)
    return y, res


def kernel(**inputs) -> np.ndarray:
    y, _ = run(inputs, trace=False)
    return y


# revision 4
# speedup vs baseline: 1.1509x; 1.0243x over previous
"""FourierBlock Trainium2 kernel, v3.

Per core (= head h), three dense matmul stages with batch-half (16-batch)
pipelining so the mid-phase overlaps the in/out HBM streams:
  fwd:  psX[(ri,m), (b16,i)] = sum_l Fwd[l,(ri,m)] q[l,(b,i)]   (K=l, 16 chunks)
  T1:   psX -> xs (cast) -> xt[(ri,i), (b16,m)]   (DVE ri=0 + PE-transpose ri=1)
  mix:  psM[(ro,o), (m,b16)] = Wbig[m]^T xt[:, (.,m)]           per mode m
  T2:   psM -> os (cast) -> o2[(ri,m), (b16,o)]   (PE transposes)
  inv:  psY[(b2,o), l] = o2-chunk^T @ G2                        per 128-col chunk

The PE clock is kept unthrottled (HAM) with prewarm matmuls and warm-keepers
gated on DMA/DVE progress. Output evacuation rotates Vector/Scalar engines.
"""

import numpy as np
import ml_dtypes

import concourse.bacc as bacc
import concourse.mybir as mybir
import concourse.tile as tile
from concourse.bass_utils import run_bass_kernel_spmd

B, L, H, E, M = 32, 2048, 8, 64, 64
NCHUNK = L // 128           # 16 l-chunks of 128
BH = B // 2                 # 16 batches per half
W = BH * E                  # 1024 columns per half
BF16 = mybir.dt.bfloat16
F32 = mybir.dt.float32
NPBF16 = ml_dtypes.bfloat16

_PROGRAM = None


def _build_program():
    nc = bacc.Bacc(target_bir_lowering=False)

    qt = nc.dram_tensor("qt", [2, NCHUNK, 128, W], BF16, kind="ExternalInput")
    wb = nc.dram_tensor("wb", [128, M * 128], BF16, kind="ExternalInput")
    fwd = nc.dram_tensor("fwd", [128, NCHUNK * 128], BF16, kind="ExternalInput")
    g2 = nc.dram_tensor("g2", [128, L], BF16, kind="ExternalInput")
    ident = nc.dram_tensor("ident", [128, 64], BF16, kind="ExternalInput")
    yt = nc.dram_tensor("yt", [2, BH // 2, 128, L], BF16, kind="ExternalOutput")

    with tile.TileContext(nc) as tc:
        with (
            tc.tile_pool(name="const", bufs=1) as cpool,
            tc.tile_pool(name="qpool", bufs=1) as qpool,
            tc.tile_pool(name="work", bufs=1) as wpool,
            tc.tile_pool(name="yout", bufs=6) as ypool,
            tc.tile_pool(name="psq", bufs=1, space="PSUM") as psq,
            tc.tile_pool(name="psm", bufs=1, space="PSUM") as psm,
            tc.tile_pool(name="psy", bufs=4, space="PSUM") as psy,
        ):
            # ---- input DMAs (issue order ~ arrival order) ----
            fwd_sb = cpool.tile([128, NCHUNK * 128], BF16, tag="fwd")
            nc.sync.dma_start(out=fwd_sb[:], in_=fwd[:])
            ident_sb = cpool.tile([128, 64], BF16, tag="ident")
            nc.sync.dma_start(out=ident_sb[:], in_=ident[:])
            wb_sb = cpool.tile([128, M * 128], BF16, tag="wb")
            nc.sync.dma_start(out=wb_sb[:, 0:4096], in_=wb[:, 0:4096])
            nc.sync.dma_start(out=wb_sb[:, 4096:8192], in_=wb[:, 4096:8192])

            q_sb = [None, None]
            for h in range(2):
                q_sb[h] = qpool.tile(
                    [128, NCHUNK * W], BF16, tag=f"q{h}", name=f"q{h}"
                )

            def dma_q(h):
                for j in range(NCHUNK // 2):
                    nc.sync.dma_start(
                        out=q_sb[h][:, j * 2 * W:(j + 1) * 2 * W].rearrange(
                            "p (c f) -> p c f", c=2
                        ),
                        in_=qt[h, 2 * j:2 * j + 2].rearrange("c p f -> p c f"),
                    )

            dma_q(0)
            g2_sb = cpool.tile([128, L], BF16, tag="g2")
            nc.sync.dma_start(out=g2_sb[:], in_=g2[:])
            dma_q(1)

            xt_sb = [None, None]
            o2_sb = [None, None]
            evac_n = [0]

            def warmmm(rhs_ap, n=1):
                # tiny matmuls that keep the PE HAM unthrottled; gated on
                # rhs_ap's producer so they spread out in time.
                k = rhs_ap.partition_size()
                b0 = rhs_ap.base_partition()
                for _ in range(n):
                    wps = psy.tile([128, 512], F32, tag="y", name="warm")
                    nc.tensor.matmul(wps[:, 0:256], fwd_sb[b0:b0 + k, 0:128],
                                     rhs_ap, start=True, stop=True)

            def fwd_stage(h):
                psX = psq.tile([128, W], F32, tag="x", name=f"psX{h}")
                for c in range(NCHUNK):
                    for j in range(2):
                        nc.tensor.matmul(
                            psX[:, j * 512:(j + 1) * 512],
                            fwd_sb[:, c * 128:(c + 1) * 128],
                            q_sb[h][:, c * W + j * 512: c * W + (j + 1) * 512],
                            start=(c == 0),
                            stop=(c == NCHUNK - 1),
                        )
                    if h == 0 and c % 2 == 1:
                        # keep HAM busy during the DMA-paced in-stream
                        warmmm(q_sb[h][:, c * W: c * W + 256], n=2)
                return psX

            def t1_stage(h, psX):
                # psX[(ri,m), (b,i)] --cast--> xs --> xt[(ri,i), (b,m)]
                # ri=0 rows on DVE (32x32 blocks), ri=1 rows on PE transpose
                xs = wpool.tile([128, W], BF16, tag=f"xs{h}", name=f"xs{h}")
                nc.scalar.copy(xs[:], psX[:])
                xt = wpool.tile([128, W], BF16, tag=f"xt{h}", name=f"xt{h}")
                xt_sb[h] = xt
                src = xs[:].rearrange("p (b i) -> p b i", i=E)
                dst = xt[:].rearrange("p (b m) -> p b m", m=E)
                # PE part: ri=1 (rows 64:128), 16 64x64 blocks
                psT1 = psm.tile([128, W], BF16, tag="m", name=f"psT1_{h}")
                for b in range(BH):
                    nc.tensor.transpose(
                        psT1[64:128, b * 64:(b + 1) * 64],
                        xs[64:128, b * 64:(b + 1) * 64],
                        ident_sb[64:128, :],
                    )
                # DVE part: ri=0 (rows 0:64), 4 strided 32x32 block calls
                for mh in range(2):
                    for ih in range(2):
                        nc.vector.transpose(
                            dst[ih * 32: ih * 32 + 32, :, mh * 32: mh * 32 + 32],
                            src[mh * 32: mh * 32 + 32, :, ih * 32: ih * 32 + 32],
                        )
                nc.vector.tensor_copy(xt[64:128, :], psT1[64:128, :])

            def mix_stage(h):
                # psM[(ro,o), (m,b)]: per-mode 16 contiguous cols (one bank)
                psM = psm.tile([128, W], F32, tag="m", name=f"psM{h}")
                xt_r = xt_sb[h][:].rearrange("p (b m) -> p m b", m=E)
                for m in range(M):
                    nc.tensor.matmul(
                        psM[:, m * BH:(m + 1) * BH],
                        wb_sb[:, m * 128:(m + 1) * 128],
                        xt_r[:, m, :],
                        start=True,
                        stop=True,
                    )
                return psM

            def t2_stage(h, psM):
                # psM[(ro,o), (m,b)] --cast--> os --PE transpose--> o2[(ri,m), (b,o)]
                os_ = wpool.tile([128, W], BF16, tag=f"os{h}", name=f"os{h}")
                nc.scalar.copy(os_[:], psM[:])
                o2 = wpool.tile([128, W], BF16, tag=f"o2{h}", name=f"o2{h}")
                o2_sb[h] = o2
                os_r = os_[:].rearrange("p (m b) -> p b m", b=BH)
                psT2 = psm.tile([128, W], BF16, tag="m", name=f"psT2_{h}")
                for ro in range(2):
                    for b in range(BH):
                        nc.tensor.transpose(
                            psT2[ro * 64:(ro + 1) * 64, b * 64:(b + 1) * 64],
                            os_r[ro * 64:(ro + 1) * 64, b, :],
                            ident_sb[ro * 64:(ro + 1) * 64, :],
                        )
                nc.vector.tensor_copy(o2[:], psT2[:])

            def inv_groups(h, ks):
                # psY[(b2,o), l] = o2[:, k-chunk]^T @ g2 ; evac + DMA out.
                # One [128,512] PSUM tile per matmul (4-deep rotation) so
                # the matmuls pipeline back-to-back instead of waiting on
                # whole-tile evacuations; the two evacs of each output DMA
                # run concurrently on Vector and Scalar.
                for k in ks:
                    lhsT = o2_sb[h][:, k * 128:(k + 1) * 128]
                    for lh in range(2):
                        ysb = ypool.tile([128, W], BF16, tag="y",
                                         name=f"ysb{h}_{k}_{lh}")
                        for j in range(2):
                            psY = psy.tile([128, 512], F32, tag="y",
                                           name=f"psY{h}_{k}_{lh}_{j}")
                            nc.tensor.matmul(
                                psY[:],
                                lhsT,
                                g2_sb[:, lh * 1024 + j * 512:
                                      lh * 1024 + (j + 1) * 512],
                                start=True,
                                stop=True,
                            )
                            if j == 0:
                                nc.vector.tensor_copy(ysb[:, 0:512], psY[:])
                            else:
                                nc.scalar.copy(ysb[:, 512:1024], psY[:])
                        nc.sync.dma_start(
                            out=yt[h, k, :, lh * 1024:(lh + 1) * 1024],
                            in_=ysb[:],
                        )

            # ---- PE program order (single in-order engine) ----
            for _ in range(16):  # prewarm: unthrottle HAM early
                warmmm(fwd_sb[:, 0:256], n=1)
            psX0 = fwd_stage(0)
            t1_stage(0, psX0)
            psM0 = mix_stage(0)
            t2_stage(0, psM0)
            psX1 = fwd_stage(1)
            t1_stage(1, psX1)
            inv_groups(0, range(0, 4))
            psM1 = mix_stage(1)
            inv_groups(0, range(4, 8))
            t2_stage(1, psM1)
            inv_groups(1, range(0, 8))

    nc.finalize()
    return nc


def _get_program():
    global _PROGRAM
    if _PROGRAM is None:
        _PROGRAM = _build_program()
    return _PROGRAM


def _host_prep(q, w_real, w_imag, index):
    q = np.asarray(q, dtype=np.float32)
    wr = np.asarray(w_real, dtype=np.float32)
    wi = np.asarray(w_imag, dtype=np.float32)
    index = np.asarray(index).astype(np.int64)

    qT = np.ascontiguousarray(q.transpose(2, 1, 0, 3))          # [H, L, B, E]
    qt4 = qT.reshape(H, NCHUNK, 128, 2, W).transpose(0, 3, 1, 2, 4)
    qt4 = np.ascontiguousarray(qt4).astype(NPBF16)              # [H, 2, 16, 128, W]

    wrT = wr.transpose(0, 1, 3, 2)                              # [h, i, m, o]
    wiT = wi.transpose(0, 1, 3, 2)
    A = np.empty((H, 128, M, 128), np.float32)
    A[:, :64, :, :64] = wrT
    A[:, :64, :, 64:] = wiT
    A[:, 64:, :, :64] = -wiT
    A[:, 64:, :, 64:] = wrT
    wb_np = A.reshape(H, 128, M * 128).astype(NPBF16)

    l = np.arange(L, dtype=np.float64)[:, None]
    ang = 2.0 * np.pi * index[None, :] * l / L                  # [L, M]
    F = np.concatenate([np.cos(ang), -np.sin(ang)], axis=1)     # [L, 2M]
    fwd_np = (
        F.reshape(NCHUNK, 128, 128).transpose(1, 0, 2).reshape(128, NCHUNK * 128)
    ).astype(NPBF16)

    mm = np.arange(M, dtype=np.float64)
    ang2 = 2.0 * np.pi * mm[:, None] * np.arange(L)[None, :] / L   # [M, L]
    c = np.where(mm == 0, 1.0, 2.0)[:, None] / L
    G2 = np.concatenate([c * np.cos(ang2), -c * np.sin(ang2)], axis=0)  # [2M, L]
    g2_np = G2.astype(NPBF16)

    ident_np = np.concatenate([np.eye(64), np.eye(64)], axis=0).astype(NPBF16)

    return qt4, wb_np, fwd_np, g2_np, ident_np


def run(inputs, trace=False):
    q = inputs["q"]
    qt4, wb_np, fwd_np, g2_np, ident_np = _host_prep(
        q, inputs["w_real"], inputs["w_imag"], inputs["index"]
    )
    nc = _get_program()
    in_maps = [
        {"qt": qt4[h], "wb": wb_np[h], "fwd": fwd_np, "g2": g2_np,
         "ident": ident_np}
        for h in range(H)
    ]
    res = run_bass_kernel_spmd(nc, in_maps, list(range(H)), trace=trace)
    arr = np.stack([res.results[h]["yt"] for h in range(H)])  # [H, 2, 8, 128, L]
    arr = arr.astype(np.float32).reshape(H, 2, BH // 2, 2, E, L)
    # p = (b2, o): b = half*16 + k*2 + b2 ; y[b, head, o, l]
    y = np.ascontiguousarray(
        arr.transpose(1, 2, 3, 0, 4, 5).reshape(B, H, E, L)
    ).astype(np.float32)
    return y, res


def kernel(**inputs) -> np.ndarray:
    y, _ = run(inputs, trace=False)
    return y


# revision 5
# speedup vs baseline: 1.1798x; 1.0251x over previous
"""FourierBlock Trainium2 kernel, v3.

Per core (= head h), three dense matmul stages with batch-half (16-batch)
pipelining so the mid-phase overlaps the in/out HBM streams:
  fwd:  psX[(ri,m), (b16,i)] = sum_l Fwd[l,(ri,m)] q[l,(b,i)]   (K=l, 16 chunks)
  T1:   psX -> xs (cast) -> xt[(ri,i), (b16,m)]   (DVE ri=0 + PE-transpose ri=1)
  mix:  psM[(ro,o), (m,b16)] = Wbig[m]^T xt[:, (.,m)]           per mode m
  T2:   psM -> os (cast) -> o2[(ri,m), (b16,o)]   (PE transposes)
  inv:  psY[(b2,o), l] = o2-chunk^T @ G2                        per 128-col chunk

The PE clock is kept unthrottled (HAM) with prewarm matmuls and warm-keepers
gated on DMA/DVE progress. Output evacuation rotates Vector/Scalar engines.
"""

import numpy as np
import ml_dtypes

import concourse.bacc as bacc
import concourse.mybir as mybir
import concourse.tile as tile
from concourse.bass_utils import run_bass_kernel_spmd

B, L, H, E, M = 32, 2048, 8, 64, 64
NCHUNK = L // 128           # 16 l-chunks of 128
BH = B // 2                 # 16 batches per half
W = BH * E                  # 1024 columns per half
BF16 = mybir.dt.bfloat16
F32 = mybir.dt.float32
NPBF16 = ml_dtypes.bfloat16

_PROGRAM = None


def _build_program():
    nc = bacc.Bacc(target_bir_lowering=False)

    qt = nc.dram_tensor("qt", [2, NCHUNK, 128, W], BF16, kind="ExternalInput")
    wb = nc.dram_tensor("wb", [128, M * 128], BF16, kind="ExternalInput")
    fwd = nc.dram_tensor("fwd", [128, NCHUNK * 128], BF16, kind="ExternalInput")
    g2 = nc.dram_tensor("g2", [128, L], BF16, kind="ExternalInput")
    ident = nc.dram_tensor("ident", [128, 64], BF16, kind="ExternalInput")
    yt = nc.dram_tensor("yt", [2, BH // 2, 128, L], BF16, kind="ExternalOutput")

    with tile.TileContext(nc) as tc:
        with (
            tc.tile_pool(name="const", bufs=1) as cpool,
            tc.tile_pool(name="qpool", bufs=1) as qpool,
            tc.tile_pool(name="work", bufs=1) as wpool,
            tc.tile_pool(name="yout", bufs=6) as ypool,
            tc.tile_pool(name="psq", bufs=1, space="PSUM") as psq,
            tc.tile_pool(name="psm", bufs=1, space="PSUM") as psm,
            tc.tile_pool(name="psy", bufs=4, space="PSUM") as psy,
        ):
            # ---- input DMAs (issue order ~ arrival order) ----
            fwd_sb = cpool.tile([128, NCHUNK * 128], BF16, tag="fwd")
            nc.sync.dma_start(out=fwd_sb[:], in_=fwd[:])
            ident_sb = cpool.tile([128, 64], BF16, tag="ident")
            nc.sync.dma_start(out=ident_sb[:], in_=ident[:])
            wb_sb = cpool.tile([128, M * 128], BF16, tag="wb")
            nc.sync.dma_start(out=wb_sb[:, 0:4096], in_=wb[:, 0:4096])
            nc.sync.dma_start(out=wb_sb[:, 4096:8192], in_=wb[:, 4096:8192])

            q_sb = [None, None]
            for h in range(2):
                q_sb[h] = qpool.tile(
                    [128, NCHUNK * W], BF16, tag=f"q{h}", name=f"q{h}"
                )

            def dma_q(h):
                for j in range(NCHUNK // 2):
                    nc.sync.dma_start(
                        out=q_sb[h][:, j * 2 * W:(j + 1) * 2 * W].rearrange(
                            "p (c f) -> p c f", c=2
                        ),
                        in_=qt[h, 2 * j:2 * j + 2].rearrange("c p f -> p c f"),
                    )

            dma_q(0)
            g2_sb = cpool.tile([128, L], BF16, tag="g2")
            nc.sync.dma_start(out=g2_sb[:], in_=g2[:])
            dma_q(1)

            xt_sb = [None, None]
            o2_sb = [None, None]
            evac_n = [0]

            def warmmm(rhs_ap, n=1):
                # tiny matmuls that keep the PE HAM unthrottled; gated on
                # rhs_ap's producer so they spread out in time.
                k = rhs_ap.partition_size()
                b0 = rhs_ap.base_partition()
                for _ in range(n):
                    wps = psy.tile([128, 512], F32, tag="y", name="warm")
                    nc.tensor.matmul(wps[:, 0:256], fwd_sb[b0:b0 + k, 0:128],
                                     rhs_ap, start=True, stop=True)

            def fwd_stage(h):
                psX = psq.tile([128, W], F32, tag="x", name=f"psX{h}")
                for c in range(NCHUNK):
                    for j in range(2):
                        nc.tensor.matmul(
                            psX[:, j * 512:(j + 1) * 512],
                            fwd_sb[:, c * 128:(c + 1) * 128],
                            q_sb[h][:, c * W + j * 512: c * W + (j + 1) * 512],
                            start=(c == 0),
                            stop=(c == NCHUNK - 1),
                        )
                    if h == 0 and c % 2 == 1:
                        # keep HAM busy during the DMA-paced in-stream
                        warmmm(q_sb[h][:, c * W: c * W + 256], n=2)
                return psX

            def t1_stage(h, psX):
                # psX[(ri,m), (b,i)] --cast--> xs --> xt[(ri,i), (b,m)]
                # ri=0 rows on DVE (32x32 blocks), ri=1 rows on PE transpose
                xs = wpool.tile([128, W], BF16, tag=f"xs{h}", name=f"xs{h}")
                nc.scalar.copy(xs[:, 0:512], psX[:, 0:512])
                nc.vector.tensor_copy(xs[:, 512:1024], psX[:, 512:1024])
                xt = wpool.tile([128, W], BF16, tag=f"xt{h}", name=f"xt{h}")
                xt_sb[h] = xt
                src = xs[:].rearrange("p (b i) -> p b i", i=E)
                dst = xt[:].rearrange("p (b m) -> p b m", m=E)
                # PE part: ri=1 (rows 64:128), 16 64x64 blocks
                psT1 = psm.tile([128, W], BF16, tag="m", name=f"psT1_{h}")
                for b in range(BH):
                    nc.tensor.transpose(
                        psT1[64:128, b * 64:(b + 1) * 64],
                        xs[64:128, b * 64:(b + 1) * 64],
                        ident_sb[64:128, :],
                    )
                # DVE part: ri=0 (rows 0:64), 4 strided 32x32 block calls
                for mh in range(2):
                    for ih in range(2):
                        nc.vector.transpose(
                            dst[ih * 32: ih * 32 + 32, :, mh * 32: mh * 32 + 32],
                            src[mh * 32: mh * 32 + 32, :, ih * 32: ih * 32 + 32],
                        )
                nc.vector.tensor_copy(xt[64:128, :], psT1[64:128, :])

            def mix_stage(h):
                # psM[(ro,o), (m,b)]: per-mode 16 contiguous cols (one bank)
                psM = psm.tile([128, W], F32, tag="m", name=f"psM{h}")
                xt_r = xt_sb[h][:].rearrange("p (b m) -> p m b", m=E)
                for m in range(M):
                    nc.tensor.matmul(
                        psM[:, m * BH:(m + 1) * BH],
                        wb_sb[:, m * 128:(m + 1) * 128],
                        xt_r[:, m, :],
                        start=True,
                        stop=True,
                    )
                return psM

            def t2_stage(h, psM):
                # psM[(ro,o), (m,b)] --cast--> os --PE transpose--> o2[(ri,m), (b,o)]
                os_ = wpool.tile([128, W], BF16, tag=f"os{h}", name=f"os{h}")
                nc.scalar.copy(os_[:, 0:512], psM[:, 0:512])
                nc.vector.tensor_copy(os_[:, 512:1024], psM[:, 512:1024])
                o2 = wpool.tile([128, W], BF16, tag=f"o2{h}", name=f"o2{h}")
                o2_sb[h] = o2
                os_r = os_[:].rearrange("p (m b) -> p b m", b=BH)
                psT2 = psm.tile([128, W], BF16, tag="m", name=f"psT2_{h}")
                for ro in range(2):
                    for b in range(BH):
                        nc.tensor.transpose(
                            psT2[ro * 64:(ro + 1) * 64, b * 64:(b + 1) * 64],
                            os_r[ro * 64:(ro + 1) * 64, b, :],
                            ident_sb[ro * 64:(ro + 1) * 64, :],
                        )
                nc.vector.tensor_copy(o2[:], psT2[:])

            def inv_groups(h, ks):
                # psY[(b2,o), l] = o2[:, k-chunk]^T @ g2 ; evac + DMA out.
                # One [128,512] PSUM tile per matmul (4-deep rotation) so
                # the matmuls pipeline back-to-back instead of waiting on
                # whole-tile evacuations; the two evacs of each output DMA
                # run concurrently on Vector and Scalar.
                for k in ks:
                    lhsT = o2_sb[h][:, k * 128:(k + 1) * 128]
                    for lh in range(2):
                        ysb = ypool.tile([128, W], BF16, tag="y",
                                         name=f"ysb{h}_{k}_{lh}")
                        for j in range(2):
                            psY = psy.tile([128, 512], F32, tag="y",
                                           name=f"psY{h}_{k}_{lh}_{j}")
                            nc.tensor.matmul(
                                psY[:],
                                lhsT,
                                g2_sb[:, lh * 1024 + j * 512:
                                      lh * 1024 + (j + 1) * 512],
                                start=True,
                                stop=True,
                            )
                            if j == 0:
                                nc.vector.tensor_copy(ysb[:, 0:512], psY[:])
                            else:
                                nc.scalar.copy(ysb[:, 512:1024], psY[:])
                        nc.sync.dma_start(
                            out=yt[h, k, :, lh * 1024:(lh + 1) * 1024],
                            in_=ysb[:],
                        )

            # ---- PE program order (single in-order engine) ----
            for _ in range(16):  # prewarm: unthrottle HAM early
                warmmm(fwd_sb[:, 0:256], n=1)
            psX0 = fwd_stage(0)
            t1_stage(0, psX0)
            psM0 = mix_stage(0)
            t2_stage(0, psM0)
            psX1 = fwd_stage(1)
            t1_stage(1, psX1)
            inv_groups(0, range(0, 4))
            psM1 = mix_stage(1)
            inv_groups(0, range(4, 8))
            t2_stage(1, psM1)
            inv_groups(1, range(0, 8))

    nc.finalize()
    return nc


def _get_program():
    global _PROGRAM
    if _PROGRAM is None:
        _PROGRAM = _build_program()
    return _PROGRAM


def _host_prep(q, w_real, w_imag, index):
    q = np.asarray(q, dtype=np.float32)
    wr = np.asarray(w_real, dtype=np.float32)
    wi = np.asarray(w_imag, dtype=np.float32)
    index = np.asarray(index).astype(np.int64)

    qT = np.ascontiguousarray(q.transpose(2, 1, 0, 3))          # [H, L, B, E]
    qt4 = qT.reshape(H, NCHUNK, 128, 2, W).transpose(0, 3, 1, 2, 4)
    qt4 = np.ascontiguousarray(qt4).astype(NPBF16)              # [H, 2, 16, 128, W]

    wrT = wr.transpose(0, 1, 3, 2)                              # [h, i, m, o]
    wiT = wi.transpose(0, 1, 3, 2)
    A = np.empty((H, 128, M, 128), np.float32)
    A[:, :64, :, :64] = wrT
    A[:, :64, :, 64:] = wiT
    A[:, 64:, :, :64] = -wiT
    A[:, 64:, :, 64:] = wrT
    wb_np = A.reshape(H, 128, M * 128).astype(NPBF16)

    l = np.arange(L, dtype=np.float64)[:, None]
    ang = 2.0 * np.pi * index[None, :] * l / L                  # [L, M]
    F = np.concatenate([np.cos(ang), -np.sin(ang)], axis=1)     # [L, 2M]
    fwd_np = (
        F.reshape(NCHUNK, 128, 128).transpose(1, 0, 2).reshape(128, NCHUNK * 128)
    ).astype(NPBF16)

    mm = np.arange(M, dtype=np.float64)
    ang2 = 2.0 * np.pi * mm[:, None] * np.arange(L)[None, :] / L   # [M, L]
    c = np.where(mm == 0, 1.0, 2.0)[:, None] / L
    G2 = np.concatenate([c * np.cos(ang2), -c * np.sin(ang2)], axis=0)  # [2M, L]
    g2_np = G2.astype(NPBF16)

    ident_np = np.concatenate([np.eye(64), np.eye(64)], axis=0).astype(NPBF16)

    return qt4, wb_np, fwd_np, g2_np, ident_np


def run(inputs, trace=False):
    q = inputs["q"]
    qt4, wb_np, fwd_np, g2_np, ident_np = _host_prep(
        q, inputs["w_real"], inputs["w_imag"], inputs["index"]
    )
    nc = _get_program()
    in_maps = [
        {"qt": qt4[h], "wb": wb_np[h], "fwd": fwd_np, "g2": g2_np,
         "ident": ident_np}
        for h in range(H)
    ]
    res = run_bass_kernel_spmd(nc, in_maps, list(range(H)), trace=trace)
    arr = np.stack([res.results[h]["yt"] for h in range(H)])  # [H, 2, 8, 128, L]
    arr = arr.astype(np.float32).reshape(H, 2, BH // 2, 2, E, L)
    # p = (b2, o): b = half*16 + k*2 + b2 ; y[b, head, o, l]
    y = np.ascontiguousarray(
        arr.transpose(1, 2, 3, 0, 4, 5).reshape(B, H, E, L)
    ).astype(np.float32)
    return y, res


def kernel(**inputs) -> np.ndarray:
    y, _ = run(inputs, trace=False)
    return y


# revision 6
# speedup vs baseline: 1.2026x; 1.0193x over previous
"""FourierBlock Trainium2 kernel, v3.

Per core (= head h), three dense matmul stages with batch-half (16-batch)
pipelining so the mid-phase overlaps the in/out HBM streams:
  fwd:  psX[(ri,m), (b16,i)] = sum_l Fwd[l,(ri,m)] q[l,(b,i)]   (K=l, 16 chunks)
  T1:   psX -> xs (cast) -> xt[(ri,i), (b16,m)]   (DVE ri=0 + PE-transpose ri=1)
  mix:  psM[(ro,o), (m,b16)] = Wbig[m]^T xt[:, (.,m)]           per mode m
  T2:   psM -> os (cast) -> o2[(ri,m), (b16,o)]   (PE transposes)
  inv:  psY[(b2,o), l] = o2-chunk^T @ G2                        per 128-col chunk

The PE clock is kept unthrottled (HAM) with prewarm matmuls and warm-keepers
gated on DMA/DVE progress. Output evacuation rotates Vector/Scalar engines.
"""

import numpy as np
import ml_dtypes

import concourse.bacc as bacc
import concourse.mybir as mybir
import concourse.tile as tile
from concourse.bass_utils import run_bass_kernel_spmd

B, L, H, E, M = 32, 2048, 8, 64, 64
NCHUNK = L // 128           # 16 l-chunks of 128
BH = B // 2                 # 16 batches per half
W = BH * E                  # 1024 columns per half
BF16 = mybir.dt.bfloat16
F32 = mybir.dt.float32
NPBF16 = ml_dtypes.bfloat16

_PROGRAM = None


def _build_program():
    nc = bacc.Bacc(target_bir_lowering=False)

    qt = nc.dram_tensor("qt", [2, NCHUNK, 128, W], BF16, kind="ExternalInput")
    wb = nc.dram_tensor("wb", [128, M * 128], BF16, kind="ExternalInput")
    fwd = nc.dram_tensor("fwd", [128, NCHUNK * 128], BF16, kind="ExternalInput")
    g2 = nc.dram_tensor("g2", [128, L], BF16, kind="ExternalInput")
    ident = nc.dram_tensor("ident", [128, 64], BF16, kind="ExternalInput")
    yt = nc.dram_tensor("yt", [2, BH // 2, 128, L], BF16, kind="ExternalOutput")

    with tile.TileContext(nc) as tc:
        with (
            tc.tile_pool(name="const", bufs=1) as cpool,
            tc.tile_pool(name="qpool", bufs=1) as qpool,
            tc.tile_pool(name="work", bufs=1) as wpool,
            tc.tile_pool(name="yout", bufs=6) as ypool,
            tc.tile_pool(name="psq", bufs=1, space="PSUM") as psq,
            tc.tile_pool(name="psm", bufs=1, space="PSUM") as psm,
            tc.tile_pool(name="psy", bufs=4, space="PSUM") as psy,
        ):
            # ---- input DMAs (issue order ~ arrival order) ----
            fwd_sb = cpool.tile([128, NCHUNK * 128], BF16, tag="fwd")
            nc.sync.dma_start(out=fwd_sb[:], in_=fwd[:])
            ident_sb = cpool.tile([128, 64], BF16, tag="ident")
            nc.sync.dma_start(out=ident_sb[:], in_=ident[:])
            wb_sb = cpool.tile([128, M * 128], BF16, tag="wb")
            nc.sync.dma_start(out=wb_sb[:, 0:4096], in_=wb[:, 0:4096])
            nc.sync.dma_start(out=wb_sb[:, 4096:8192], in_=wb[:, 4096:8192])

            q_sb = [None, None]
            for h in range(2):
                q_sb[h] = qpool.tile(
                    [128, NCHUNK * W], BF16, tag=f"q{h}", name=f"q{h}"
                )

            def dma_q(h):
                for j in range(NCHUNK // 2):
                    nc.sync.dma_start(
                        out=q_sb[h][:, j * 2 * W:(j + 1) * 2 * W].rearrange(
                            "p (c f) -> p c f", c=2
                        ),
                        in_=qt[h, 2 * j:2 * j + 2].rearrange("c p f -> p c f"),
                    )

            dma_q(0)
            g2_sb = cpool.tile([128, L], BF16, tag="g2")
            nc.sync.dma_start(out=g2_sb[:], in_=g2[:])
            dma_q(1)

            xt_sb = [None, None]
            o2_sb = [None, None]
            evac_n = [0]

            def warmmm(rhs_ap, n=1):
                # tiny matmuls that keep the PE HAM unthrottled; gated on
                # rhs_ap's producer so they spread out in time.
                k = rhs_ap.partition_size()
                b0 = rhs_ap.base_partition()
                for _ in range(n):
                    wps = psy.tile([128, 512], F32, tag="y", name="warm")
                    nc.tensor.matmul(wps[:, 0:256], fwd_sb[b0:b0 + k, 0:128],
                                     rhs_ap, start=True, stop=True)

            def fwd_stage(h):
                psX = psq.tile([128, W], F32, tag="x", name=f"psX{h}")
                for c in range(NCHUNK):
                    for j in range(2):
                        nc.tensor.matmul(
                            psX[:, j * 512:(j + 1) * 512],
                            fwd_sb[:, c * 128:(c + 1) * 128],
                            q_sb[h][:, c * W + j * 512: c * W + (j + 1) * 512],
                            start=(c == 0),
                            stop=(c == NCHUNK - 1),
                        )
                    if h == 0 and c % 2 == 1:
                        # keep HAM busy during the DMA-paced in-stream
                        warmmm(q_sb[h][:, c * W: c * W + 256], n=2)
                return psX

            def t1_stage(h, psX):
                # psX[(ri,m), (b,i)] --cast--> xs --> xt[(ri,i), (b,m)]
                # ri=0 rows on DVE (32x32 blocks), ri=1 rows on PE transpose
                xs = wpool.tile([128, W], BF16, tag=f"xs{h}", name=f"xs{h}")
                nc.scalar.copy(xs[:, 0:512], psX[:, 0:512])
                nc.vector.tensor_copy(xs[:, 512:1024], psX[:, 512:1024])
                xt = wpool.tile([128, W], BF16, tag=f"xt{h}", name=f"xt{h}")
                xt_sb[h] = xt
                src = xs[:].rearrange("p (b i) -> p b i", i=E)
                dst = xt[:].rearrange("p (b m) -> p b m", m=E)
                # PE part: ri=1 (rows 64:128), 16 64x64 blocks
                psT1 = psm.tile([128, W], BF16, tag="m", name=f"psT1_{h}")
                for b in range(BH):
                    nc.tensor.transpose(
                        psT1[64:128, b * 64:(b + 1) * 64],
                        xs[64:128, b * 64:(b + 1) * 64],
                        ident_sb[64:128, :],
                    )
                # DVE part: ri=0 (rows 0:64), 4 strided 32x32 block calls
                for mh in range(2):
                    for ih in range(2):
                        nc.vector.transpose(
                            dst[ih * 32: ih * 32 + 32, :, mh * 32: mh * 32 + 32],
                            src[mh * 32: mh * 32 + 32, :, ih * 32: ih * 32 + 32],
                        )
                nc.vector.tensor_copy(xt[64:128, 0:512], psT1[64:128, 0:512])
                nc.scalar.copy(xt[64:128, 512:1024], psT1[64:128, 512:1024])

            def mix_stage(h):
                # psM[(ro,o), (m,b)]: per-mode 16 contiguous cols (one bank)
                psM = psm.tile([128, W], F32, tag="m", name=f"psM{h}")
                xt_r = xt_sb[h][:].rearrange("p (b m) -> p m b", m=E)
                for m in range(M):
                    nc.tensor.matmul(
                        psM[:, m * BH:(m + 1) * BH],
                        wb_sb[:, m * 128:(m + 1) * 128],
                        xt_r[:, m, :],
                        start=True,
                        stop=True,
                    )
                return psM

            def t2_stage(h, psM):
                # psM[(ro,o), (m,b)] --cast--> os --PE transpose--> o2[(ri,m), (b,o)]
                os_ = wpool.tile([128, W], BF16, tag=f"os{h}", name=f"os{h}")
                nc.scalar.copy(os_[:, 0:512], psM[:, 0:512])
                nc.vector.tensor_copy(os_[:, 512:1024], psM[:, 512:1024])
                o2 = wpool.tile([128, W], BF16, tag=f"o2{h}", name=f"o2{h}")
                o2_sb[h] = o2
                os_r = os_[:].rearrange("p (m b) -> p b m", b=BH)
                psT2 = psm.tile([128, W], BF16, tag="m", name=f"psT2_{h}")
                for ro in range(2):
                    for b in range(BH):
                        nc.tensor.transpose(
                            psT2[ro * 64:(ro + 1) * 64, b * 64:(b + 1) * 64],
                            os_r[ro * 64:(ro + 1) * 64, b, :],
                            ident_sb[ro * 64:(ro + 1) * 64, :],
                        )
                nc.vector.tensor_copy(o2[:, 0:512], psT2[:, 0:512])
                nc.scalar.copy(o2[:, 512:1024], psT2[:, 512:1024])

            def inv_groups(h, ks):
                # psY[(b2,o), l] = o2[:, k-chunk]^T @ g2 ; evac + DMA out.
                # One [128,512] PSUM tile per matmul (4-deep rotation) so
                # the matmuls pipeline back-to-back instead of waiting on
                # whole-tile evacuations; the two evacs of each output DMA
                # run concurrently on Vector and Scalar.
                for k in ks:
                    lhsT = o2_sb[h][:, k * 128:(k + 1) * 128]
                    for lh in range(2):
                        ysb = ypool.tile([128, W], BF16, tag="y",
                                         name=f"ysb{h}_{k}_{lh}")
                        for j in range(2):
                            psY = psy.tile([128, 512], F32, tag="y",
                                           name=f"psY{h}_{k}_{lh}_{j}")
                            nc.tensor.matmul(
                                psY[:],
                                lhsT,
                                g2_sb[:, lh * 1024 + j * 512:
                                      lh * 1024 + (j + 1) * 512],
                                start=True,
                                stop=True,
                            )
                            if j == 0:
                                nc.vector.tensor_copy(ysb[:, 0:512], psY[:])
                            else:
                                nc.scalar.copy(ysb[:, 512:1024], psY[:])
                        nc.sync.dma_start(
                            out=yt[h, k, :, lh * 1024:(lh + 1) * 1024],
                            in_=ysb[:],
                        )

            # ---- PE program order (single in-order engine) ----
            for _ in range(16):  # prewarm: unthrottle HAM early
                warmmm(fwd_sb[:, 0:256], n=1)
            psX0 = fwd_stage(0)
            t1_stage(0, psX0)
            psM0 = mix_stage(0)
            t2_stage(0, psM0)
            psX1 = fwd_stage(1)
            t1_stage(1, psX1)
            inv_groups(0, range(0, 4))
            psM1 = mix_stage(1)
            inv_groups(0, range(4, 6))
            t2_stage(1, psM1)
            inv_groups(0, range(6, 8))
            inv_groups(1, range(0, 8))

    nc.finalize()
    return nc


def _get_program():
    global _PROGRAM
    if _PROGRAM is None:
        _PROGRAM = _build_program()
    return _PROGRAM


def _host_prep(q, w_real, w_imag, index):
    q = np.asarray(q, dtype=np.float32)
    wr = np.asarray(w_real, dtype=np.float32)
    wi = np.asarray(w_imag, dtype=np.float32)
    index = np.asarray(index).astype(np.int64)

    qT = np.ascontiguousarray(q.transpose(2, 1, 0, 3))          # [H, L, B, E]
    qt4 = qT.reshape(H, NCHUNK, 128, 2, W).transpose(0, 3, 1, 2, 4)
    qt4 = np.ascontiguousarray(qt4).astype(NPBF16)              # [H, 2, 16, 128, W]

    wrT = wr.transpose(0, 1, 3, 2)                              # [h, i, m, o]
    wiT = wi.transpose(0, 1, 3, 2)
    A = np.empty((H, 128, M, 128), np.float32)
    A[:, :64, :, :64] = wrT
    A[:, :64, :, 64:] = wiT
    A[:, 64:, :, :64] = -wiT
    A[:, 64:, :, 64:] = wrT
    wb_np = A.reshape(H, 128, M * 128).astype(NPBF16)

    l = np.arange(L, dtype=np.float64)[:, None]
    ang = 2.0 * np.pi * index[None, :] * l / L                  # [L, M]
    F = np.concatenate([np.cos(ang), -np.sin(ang)], axis=1)     # [L, 2M]
    fwd_np = (
        F.reshape(NCHUNK, 128, 128).transpose(1, 0, 2).reshape(128, NCHUNK * 128)
    ).astype(NPBF16)

    mm = np.arange(M, dtype=np.float64)
    ang2 = 2.0 * np.pi * mm[:, None] * np.arange(L)[None, :] / L   # [M, L]
    c = np.where(mm == 0, 1.0, 2.0)[:, None] / L
    G2 = np.concatenate([c * np.cos(ang2), -c * np.sin(ang2)], axis=0)  # [2M, L]
    g2_np = G2.astype(NPBF16)

    ident_np = np.concatenate([np.eye(64), np.eye(64)], axis=0).astype(NPBF16)

    return qt4, wb_np, fwd_np, g2_np, ident_np


def run(inputs, trace=False):
    q = inputs["q"]
    qt4, wb_np, fwd_np, g2_np, ident_np = _host_prep(
        q, inputs["w_real"], inputs["w_imag"], inputs["index"]
    )
    nc = _get_program()
    in_maps = [
        {"qt": qt4[h], "wb": wb_np[h], "fwd": fwd_np, "g2": g2_np,
         "ident": ident_np}
        for h in range(H)
    ]
    res = run_bass_kernel_spmd(nc, in_maps, list(range(H)), trace=trace)
    arr = np.stack([res.results[h]["yt"] for h in range(H)])  # [H, 2, 8, 128, L]
    arr = arr.astype(np.float32).reshape(H, 2, BH // 2, 2, E, L)
    # p = (b2, o): b = half*16 + k*2 + b2 ; y[b, head, o, l]
    y = np.ascontiguousarray(
        arr.transpose(1, 2, 3, 0, 4, 5).reshape(B, H, E, L)
    ).astype(np.float32)
    return y, res


def kernel(**inputs) -> np.ndarray:
    y, _ = run(inputs, trace=False)
    return y
